# revision 2
# baseline (speedup 1.0000x reference)
"""GCNConv on 8 Trainium2 NeuronCores (Bass/Tile, SPMD).

out = D^-1/2 (A+I) D^-1/2 (X @ W.T),   deg = in-degree(col) + 1

Formulation (exact in real arithmetic):
    agg[:, r] = sum_{e: dst=r} d[col_e] * X[col_e]   + d[r] * X[r]
    out[r]    = d[r] * (agg[:, r] @ W.T)             (d = deg^-1/2)

Distribution: destinations (rows) sharded across 8 cores (12500 each);
each core processes edges whose destination lands in its shard. X (bf16)
and W are replicated.

Per-core device algorithm (SPMD; per-core index tables padded to a
common max-over-cores structure):
  * Edge slots grouped by (range of RNG_DTS dest-tiles, source chunk of
    25000 rows), sorted by destination, packed densely at 128 slots/tile.
  * One dma_gather (int16 chunk-relative indices) per segment pulls the
    256-byte bf16 X rows; trailing pads use index -1 + a per-core count
    register so padding costs no descriptors. The gather dominates the
    runtime (~4 ns/descriptor marginal on the single SWDGE queue).
  * Routing: per 128-slot tile, a host-built selection matrix S
    (bf16, carrying the d[col] edge values) scatters slots to
    destinations on the PE: psum[feat, dest] += g_tile.T @ S_tile,
    accumulated over a range-wide one-bank PSUM tile [128, 512].
  * Self-loops skip the gather: their rows are the core's own X shard,
    streamed densely per range (static DMA) and routed with diagonal-S
    tiles carrying d[dest].
  * Finalize per dest-tile: matmul with W.T (contraction over features),
    scale by d[dest], DMA out.

Host does index marshaling only (bucketing, sorting, degree counts, S
structure); all floating-point math on X/W runs on device (the bf16
casts of X/S-values are rounding, not arithmetic).
"""

import math

import numpy as np
import ml_dtypes

import concourse.bacc as bacc
import concourse.mybir as mybir
import concourse.tile as tile
from concourse.bass_utils import run_bass_kernel_spmd
from concourse import library_config

NCORES = 8
P = 128
CH_SPAN = 25000          # source rows per gather chunk (int16-indexable)
RNG_DTS = 4              # dest-tiles per range (psum tile = 1 bank)

F32 = mybir.dt.float32
BF16 = mybir.dt.bfloat16
I16 = mybir.dt.int16


class Plan:
    pass


# ----------------------------------------------------------------------------
# Host-side index marshaling
# ----------------------------------------------------------------------------

def _preprocess(edge_index: np.ndarray, n_nodes: int):
    ns = n_nodes // NCORES
    rt = math.ceil(ns / P)
    nch = math.ceil(n_nodes / CH_SPAN)
    nrng = math.ceil(rt / RNG_DTS)
    rw = RNG_DTS * P

    row = np.asarray(edge_index[0]).astype(np.int64)
    col = np.asarray(edge_index[1]).astype(np.int64)
    deg = (np.bincount(col, minlength=n_nodes) + 1).astype(np.float32)
    dinv = deg ** -0.5

    core = row // ns
    cores = []
    for m in range(NCORES):
        sel = core == m
        r_l = row[sel] - m * ns
        c_g = col[sel]
        rg = r_l // rw
        ch = np.minimum(c_g // CH_SPAN, nch - 1)
        order = np.lexsort((c_g, r_l, ch, rg))
        r_l, c_g = r_l[order], c_g[order]
        code = rg[order] * nch + ch[order]
        bounds = np.searchsorted(code, np.arange(nrng * nch + 1))
        cores.append(dict(r_l=r_l, c_g=c_g, bounds=bounds))

    plan = Plan()
    plan.ns, plan.rt, plan.nch, plan.nrng = ns, rt, nch, nrng
    plan.segs = []          # gather segments
    plan.tiles = []         # all S tiles in stream order
    jtot = 0
    for rg in range(nrng):
        ndt = min(RNG_DTS, rt - rg * RNG_DTS)
        for t in range(ndt):
            plan.tiles.append(dict(kind="self", rng=rg, t=t,
                                   dmin=t * P, nd=P))
        for c in range(nch):
            g = rg * nch + c
            ntiles = 0
            for m in range(NCORES):
                b = cores[m]["bounds"]
                ntiles = max(ntiles, (int(b[g + 1] - b[g]) + P - 1) // P)
            if ntiles == 0:
                continue
            seg = dict(base=c * CH_SPAN, t16_0=jtot * 8, n=ntiles * P,
                       j0=jtot, ntiles=ntiles, rng=rg, c=c, g=g)
            plan.segs.append(seg)
            for jj in range(ntiles):
                plan.tiles.append(dict(kind="edge", rng=rg, seg=seg, jj=jj,
                                       j=jtot + jj))
            jtot += ntiles
    plan.jtot = jtot
    plan.tot16 = jtot * 8

    nslots = jtot * P
    gidx = np.zeros((NCORES, P, max(1, plan.tot16)), np.int16)
    dval = np.zeros((NCORES, nslots), np.float32)
    dest_arr = np.full((NCORES, nslots), -1, np.int64)
    cnts = np.zeros((NCORES, max(1, len(plan.segs))), np.int32)
    for m in range(NCORES):
        r_l, c_g, b = cores[m]["r_l"], cores[m]["c_g"], cores[m]["bounds"]
        idx16 = np.full(nslots, -1, np.int16)
        for si, seg in enumerate(plan.segs):
            g = seg["g"]
            lo, hi = int(b[g]), int(b[g + 1])
            n = hi - lo
            if n == 0:
                # still need >= 1 valid index (dummy row 0, zero S row)
                idx16[seg["j0"] * P] = 0
                cnts[m, si] = 1
                continue
            cnts[m, si] = n
            s0 = seg["j0"] * P
            cg = c_g[lo:hi]
            idx16[s0:s0 + n] = (cg - seg["base"]).astype(np.int16)
            dval[m, s0:s0 + n] = dinv[cg]
            dest_arr[m, s0:s0 + n] = r_l[lo:hi] - seg["rng"] * rw
        w = idx16.reshape(max(1, plan.tot16), 16).T
        gidx[m] = np.tile(w, (8, 1))

    # per-tile dest windows (edge tiles: union over cores)
    da = dest_arr.reshape(NCORES, jtot, P)
    da_min = np.where(da < 0, 10 ** 9, da).min(axis=(0, 2))
    da_max = da.max(axis=(0, 2))
    for ti in plan.tiles:
        if ti["kind"] == "edge":
            j = ti["j"]
            ti["dmin"] = int(da_min[j])
            ti["nd"] = int(da_max[j] - da_min[j] + 1)
            assert 1 <= ti["nd"] <= rw
    s0 = 0
    for ti in plan.tiles:
        ti["s0"] = s0
        s0 += ti["nd"]
    plan.stot = s0

    # S matrices (bf16, carrying d[col] / diagonal self d values)
    s_pack = np.zeros((NCORES, P, plan.stot), ml_dtypes.bfloat16)
    ar = np.arange(P)
    dv = dval.reshape(NCORES, jtot, P)
    for m in range(NCORES):
        for ti in plan.tiles:
            if ti["kind"] == "self":
                dt = ti["rng"] * RNG_DTS + ti["t"]
                gl = m * ns + dt * P + ar
                valid = gl < (m + 1) * ns
                blk = np.zeros((P, P), np.float32)
                blk[ar[valid], ar[valid]] = dinv[
                    np.minimum(gl, n_nodes - 1)][valid]
                s_pack[m, :, ti["s0"]:ti["s0"] + P] = blk
            else:
                j = ti["j"]
                dl = da[m, j]
                v = dl >= 0
                if not v.any():
                    continue
                blk = np.zeros((P, ti["nd"]), np.float32)
                blk[ar[v], dl[v] - ti["dmin"]] = dv[m, j][v]
                s_pack[m, :, ti["s0"]:ti["s0"] + ti["nd"]] = blk

    deg_nat = np.full((NCORES, P, rt), 1.0, np.float32)
    for m in range(NCORES):
        d = np.full(rt * P, 1.0, np.float32)
        d[:ns] = deg[m * ns:(m + 1) * ns]
        deg_nat[m] = d.reshape(rt, P).T

    plan.rng_scols = []
    for rg in range(nrng):
        ts = [ti for ti in plan.tiles if ti["rng"] == rg]
        plan.rng_scols.append((ts[0]["s0"], ts[-1]["s0"] + ts[-1]["nd"])
                              if ts else (0, 0))
    plan.swmax = max((b - a for a, b in plan.rng_scols), default=1)
    plan.nmax = max((s["n"] for s in plan.segs), default=P)

    data = dict(gidx=gidx, s_pack=s_pack, deg_nat=deg_nat, cnts=cnts)
    return plan, data


# ----------------------------------------------------------------------------
# Device program (identical for all cores)
# ----------------------------------------------------------------------------

def _build_nc(n_nodes: int, plan: Plan):
    ns, rt, nch, nrng = plan.ns, plan.rt, plan.nch, plan.nrng
    nc = bacc.Bacc("TRN2", target_bir_lowering=False, debug=False,
                   num_devices=NCORES)

    x_d = nc.dram_tensor("x", [n_nodes, P], BF16, kind="ExternalInput").ap()
    wt_d = nc.dram_tensor("wt", [P, P], F32, kind="ExternalInput").ap()
    gix_d = nc.dram_tensor("gidx", [P, max(1, plan.tot16)], I16,
                           kind="ExternalInput").ap()
    dnat_d = nc.dram_tensor("deg_nat", [P, rt], F32,
                            kind="ExternalInput").ap()
    s_d = nc.dram_tensor("s_pack", [P, plan.stot], BF16,
                         kind="ExternalInput").ap()
    cnt_d = nc.dram_tensor("cnts", [1, max(1, len(plan.segs))],
                           mybir.dt.int32, kind="ExternalInput").ap()
    xs_d = nc.dram_tensor("xself", [nrng * RNG_DTS * P, P], BF16,
                          kind="ExternalInput").ap()
    out_d = nc.dram_tensor("out", [rt * P, P], F32, kind="ExternalOutput").ap()

    pw = RNG_DTS * P
    with tile.TileContext(nc) as tc:
        nc.gpsimd.load_library(library_config.mlp)
        with (
            tc.tile_pool(name="const", bufs=1) as cpool,
            tc.tile_pool(name="gbuf", bufs=3) as gpool,
            tc.tile_pool(name="sfbuf", bufs=2) as sfpool,
            tc.tile_pool(name="sbuf_s", bufs=2) as spool,
            tc.tile_pool(name="fin", bufs=4) as fpool,
            tc.tile_pool(name="pacc", bufs=4, space="PSUM") as papool,
            tc.tile_pool(name="pout", bufs=2, space="PSUM") as popool,
        ):
            wt_sb = cpool.tile([P, P], F32)
            nc.sync.dma_start(out=wt_sb[:], in_=wt_d[:, :])
            gidx_sb = cpool.tile([P, max(1, plan.tot16)], I16)
            nc.sync.dma_start(out=gidx_sb[:], in_=gix_d[:, :])

            dnat_sb = cpool.tile([P, rt], F32)
            nc.sync.dma_start(out=dnat_sb[:], in_=dnat_d[:, :])
            nc.scalar.activation(dnat_sb[:], dnat_sb[:],
                                 mybir.ActivationFunctionType.Sqrt)
            d_nat = cpool.tile([P, rt], F32)
            nc.vector.reciprocal(d_nat[:], dnat_sb[:])

            zcol = cpool.tile([1, P], BF16)
            nc.vector.memset(zcol[:], 0.0)
            zrow = cpool.tile([1, pw], BF16)
            nc.vector.memset(zrow[:], 0.0)

            cnt_sb = cpool.tile([1, max(1, len(plan.segs))], mybir.dt.int32)
            nc.sync.dma_start(out=cnt_sb[:], in_=cnt_d[:, :])
            cnt_regs = [nc.gpsimd.alloc_register(f"cntr{i}") for i in range(4)]

            self_tiles = {}
            edge_tiles = {}
            for ti in plan.tiles:
                d = self_tiles if ti["kind"] == "self" else edge_tiles
                d.setdefault(ti["rng"], []).append(ti)
            segs_by_rng = {}
            for si, seg in enumerate(plan.segs):
                segs_by_rng.setdefault(seg["rng"], []).append((si, seg))

            for rg in range(nrng):
                sw0, sw1 = plan.rng_scols[rg]
                s_sb = spool.tile([P, plan.swmax], BF16, tag="s_sb")
                nc.sync.dma_start(out=s_sb[:, :sw1 - sw0],
                                  in_=s_d[:, sw0:sw1])

                pt = papool.tile([P, pw], F32, tag="pacc")
                nmm = len(self_tiles.get(rg, [])) + sum(
                    seg["ntiles"] for _, seg in segs_by_rng.get(rg, []))
                nc.tensor.matmul(pt[:], lhsT=zcol[:], rhs=zrow[:],
                                 start=True, stop=(nmm == 0),
                                 skip_group_check=True)
                k = 0

                def mm(lhs, ti):
                    nonlocal k
                    dmin, nd = ti["dmin"], ti["nd"]
                    sa = ti["s0"] - sw0
                    nc.tensor.matmul(
                        pt[:, dmin:dmin + nd], lhsT=lhs,
                        rhs=s_sb[:, sa:sa + nd],
                        start=False, stop=(k == nmm - 1),
                        skip_group_check=True,
                    )
                    k += 1

                sf = sfpool.tile([P, RNG_DTS * P], BF16, tag="sf")
                nc.sync.dma_start(
                    out=sf[:].rearrange("p (t f) -> p t f", f=P),
                    in_=xs_d[rg * pw:(rg + 1) * pw, :].rearrange(
                        "(t p) f -> p t f", p=P))
                for ti in self_tiles.get(rg, []):
                    mm(sf[:, ti["t"] * P:(ti["t"] + 1) * P], ti)

                et = edge_tiles.get(rg, [])
                ei = 0
                for si, seg in segs_by_rng.get(rg, []):
                    jseg, nseg = seg["ntiles"], seg["n"]
                    g = gpool.tile([P, plan.nmax], BF16, tag="g")
                    g3 = g[:, :nseg].rearrange("p (j f) -> p j f", f=P)
                    # pad slots are skipped by the gather (idx -1); zero them
                    # so the matmuls see no stale garbage
                    nc.vector.memset(g[:, :nseg], 0.0)
                    span = min(CH_SPAN, n_nodes - seg["base"])
                    creg = cnt_regs[si % len(cnt_regs)]
                    nc.gpsimd.reg_load(creg, cnt_sb[0:1, si:si + 1])
                    nc.gpsimd.dma_gather(
                        g3, x_d[seg["base"]:seg["base"] + span, :],
                        gidx_sb[:, seg["t16_0"]:seg["t16_0"] + jseg * 8],
                        nseg, creg, P, single_packet=False,
                    )
                    for jj in range(jseg):
                        ti = et[ei]
                        assert ti["seg"] is seg and ti["jj"] == jj
                        mm(g[:, jj * P:(jj + 1) * P], ti)
                        ei += 1

                for dl in range(min(RNG_DTS, rt - rg * RNG_DTS)):
                    dt = rg * RNG_DTS + dl
                    aggt = fpool.tile([P, P], F32, tag="aggt")
                    nc.vector.tensor_copy(aggt[:], pt[:, dl * P:(dl + 1) * P])
                    op = popool.tile([P, P], F32, tag="op")
                    nc.tensor.matmul(op[:], lhsT=aggt[:], rhs=wt_sb[:],
                                     start=True, stop=True)
                    ob = fpool.tile([P, P], F32, tag="ob")
                    nc.vector.tensor_scalar_mul(ob[:], op[:],
                                                d_nat[:, dt:dt + 1])
                    nc.sync.dma_start(out=out_d[dt * P:(dt + 1) * P, :],
                                      in_=ob[:])
    nc.compile()
    return nc


# ----------------------------------------------------------------------------
# Entry point
# ----------------------------------------------------------------------------

_CACHE: dict = {}


def _prepare(X, W, edge_index):
    X = np.ascontiguousarray(np.asarray(X, dtype=np.float32))
    W = np.asarray(W, dtype=np.float32)
    edge_index = np.asarray(edge_index)
    n = X.shape[0]
    plan, data = _preprocess(edge_index, n)
    key = (n, plan.jtot, plan.stot, tuple(s["n"] for s in plan.segs))
    if key not in _CACHE:
        _CACHE.clear()
        _CACHE[key] = _build_nc(n, plan)
    nc = _CACHE[key]
    wt = np.ascontiguousarray(W.T)
    xs = X.astype(ml_dtypes.bfloat16)
    in_maps = []
    for m in range(NCORES):
        xsf = np.zeros((plan.nrng * RNG_DTS * P, P), ml_dtypes.bfloat16)
        xsf[:plan.ns] = xs[m * plan.ns:(m + 1) * plan.ns]
        in_maps.append({
            "x": np.ascontiguousarray(xs),
            "wt": wt,
            "gidx": np.ascontiguousarray(data["gidx"][m]),
            "deg_nat": np.ascontiguousarray(data["deg_nat"][m]),
            "s_pack": np.ascontiguousarray(data["s_pack"][m]),
            "cnts": np.ascontiguousarray(data["cnts"][m][None, :]),
            "xself": xsf,
        })
    return nc, in_maps, plan


def kernel(X, W, edge_index):
    nc, in_maps, plan = _prepare(X, W, edge_index)
    res = run_bass_kernel_spmd(nc, in_maps, core_ids=list(range(NCORES)))
    ns = plan.ns
    return np.concatenate([res.results[m]["out"][:ns] for m in range(NCORES)],
                          axis=0)


# revision 4
# speedup vs baseline: 1.5976x; 1.5976x over previous
"""GCNConv on 8 Trainium2 NeuronCores (Bass/Tile, SPMD).

out = D^-1/2 (A+I) D^-1/2 (X @ W.T),   deg = in-degree(col) + 1

Formulation (exact in real arithmetic):
    agg[:, r] = sum_{e: dst=r} d[col_e] * X[col_e]   + d[r] * X[r]
    out[r]    = d[r] * (agg[:, r] @ W.T)             (d = deg^-1/2)

Distribution: destinations (rows) sharded across 8 cores (12500 each);
each core processes edges whose destination lands in its shard. X (bf16)
and W are replicated.

Per-core device algorithm (SPMD; per-core index tables padded to a
common max-over-cores structure):
  * Edge slots grouped by (range of RNG_DTS dest-tiles, source chunk of
    25000 rows), sorted by destination, packed densely at 128 slots/tile.
  * One dma_gather (int16 chunk-relative indices) per segment pulls the
    256-byte bf16 X rows; trailing pads use index -1 + a per-core count
    register so padding costs no descriptors. The gather dominates the
    runtime (~4 ns/descriptor marginal on the single SWDGE queue).
  * Routing: per 128-slot tile, a host-built selection matrix S
    (bf16, carrying the d[col] edge values) scatters slots to
    destinations on the PE: psum[feat, dest] += g_tile.T @ S_tile,
    accumulated over a range-wide one-bank PSUM tile [128, 512].
  * Self-loops skip the gather: their rows are the core's own X shard,
    streamed densely per range (static DMA) and routed with diagonal-S
    tiles carrying d[dest].
  * Finalize per dest-tile: matmul with W.T (contraction over features),
    scale by d[dest], DMA out.

Host does index marshaling only (bucketing, sorting, degree counts, S
structure); all floating-point math on X/W runs on device (the bf16
casts of X/S-values are rounding, not arithmetic).
"""

import math

import numpy as np
import ml_dtypes

import concourse.bacc as bacc
import concourse.mybir as mybir
import concourse.tile as tile
from concourse.bass_utils import run_bass_kernel_spmd
from concourse import library_config

NCORES = 8
P = 128
CH_SPAN = 25000          # source rows per gather chunk (int16-indexable)
RNG_DTS = 4              # dest-tiles per range (psum tile = 1 bank)

F32 = mybir.dt.float32
BF16 = mybir.dt.bfloat16
I16 = mybir.dt.int16


class Plan:
    pass


# ----------------------------------------------------------------------------
# Host-side index marshaling
# ----------------------------------------------------------------------------

def _preprocess(edge_index: np.ndarray, n_nodes: int):
    ns = n_nodes // NCORES
    rt = math.ceil(ns / P)
    nch = math.ceil(n_nodes / CH_SPAN)
    nrng = math.ceil(rt / RNG_DTS)
    rw = RNG_DTS * P

    row = np.asarray(edge_index[0]).astype(np.int64)
    col = np.asarray(edge_index[1]).astype(np.int64)
    deg = (np.bincount(col, minlength=n_nodes) + 1).astype(np.float32)
    dinv = deg ** -0.5

    core = row // ns
    cores = []
    for m in range(NCORES):
        sel = core == m
        r_l = row[sel] - m * ns
        c_g = col[sel]
        rg = r_l // rw
        ch = np.minimum(c_g // CH_SPAN, nch - 1)
        order = np.lexsort((c_g, r_l, ch, rg))
        r_l, c_g = r_l[order], c_g[order]
        code = rg[order] * nch + ch[order]
        bounds = np.searchsorted(code, np.arange(nrng * nch + 1))
        cores.append(dict(r_l=r_l, c_g=c_g, bounds=bounds))

    plan = Plan()
    plan.ns, plan.rt, plan.nch, plan.nrng = ns, rt, nch, nrng
    plan.segs = []          # gather segments
    plan.tiles = []         # all S tiles in stream order
    jtot = 0
    for rg in range(nrng):
        ndt = min(RNG_DTS, rt - rg * RNG_DTS)
        for t in range(ndt):
            plan.tiles.append(dict(kind="self", rng=rg, t=t,
                                   dmin=t * P, nd=P))
        for c in range(nch):
            g = rg * nch + c
            ntiles = 0
            for m in range(NCORES):
                b = cores[m]["bounds"]
                ntiles = max(ntiles, (int(b[g + 1] - b[g]) + P - 1) // P)
            if ntiles == 0:
                continue
            seg = dict(base=c * CH_SPAN, t16_0=jtot * 8, n=ntiles * P,
                       j0=jtot, ntiles=ntiles, rng=rg, c=c, g=g)
            plan.segs.append(seg)
            for jj in range(ntiles):
                plan.tiles.append(dict(kind="edge", rng=rg, seg=seg, jj=jj,
                                       j=jtot + jj))
            jtot += ntiles
    plan.jtot = jtot
    plan.tot16 = jtot * 8

    nslots = jtot * P
    gidx = np.zeros((NCORES, P, max(1, plan.tot16)), np.int16)
    dval = np.zeros((NCORES, nslots), np.float32)
    dest_arr = np.full((NCORES, nslots), -1, np.int64)
    cnts = np.zeros((NCORES, max(1, len(plan.segs))), np.int32)
    for m in range(NCORES):
        r_l, c_g, b = cores[m]["r_l"], cores[m]["c_g"], cores[m]["bounds"]
        idx16 = np.full(nslots, -1, np.int16)
        for si, seg in enumerate(plan.segs):
            g = seg["g"]
            lo, hi = int(b[g]), int(b[g + 1])
            n = hi - lo
            if n == 0:
                # still need >= 1 valid index (dummy row 0, zero S row)
                idx16[seg["j0"] * P] = 0
                cnts[m, si] = 1
                continue
            cnts[m, si] = n
            s0 = seg["j0"] * P
            cg = c_g[lo:hi]
            idx16[s0:s0 + n] = (cg - seg["base"]).astype(np.int16)
            dval[m, s0:s0 + n] = dinv[cg]
            dest_arr[m, s0:s0 + n] = r_l[lo:hi] - seg["rng"] * rw
        w = idx16.reshape(max(1, plan.tot16), 16).T
        gidx[m] = np.tile(w, (8, 1))

    # per-tile dest windows (edge tiles: union over cores)
    da = dest_arr.reshape(NCORES, jtot, P)
    da_min = np.where(da < 0, 10 ** 9, da).min(axis=(0, 2))
    da_max = da.max(axis=(0, 2))
    for ti in plan.tiles:
        if ti["kind"] == "edge":
            j = ti["j"]
            ti["dmin"] = int(da_min[j])
            ti["nd"] = int(da_max[j] - da_min[j] + 1)
            assert 1 <= ti["nd"] <= rw
    s0 = 0
    for ti in plan.tiles:
        ti["s0"] = s0
        s0 += ti["nd"]
    plan.stot = s0

    # S matrices (bf16, carrying d[col] / diagonal self d values)
    s_pack = np.zeros((NCORES, P, plan.stot), ml_dtypes.bfloat16)
    ar = np.arange(P)
    dv = dval.reshape(NCORES, jtot, P)
    for m in range(NCORES):
        for ti in plan.tiles:
            if ti["kind"] == "self":
                dt = ti["rng"] * RNG_DTS + ti["t"]
                gl = m * ns + dt * P + ar
                valid = gl < (m + 1) * ns
                blk = np.zeros((P, P), np.float32)
                blk[ar[valid], ar[valid]] = dinv[
                    np.minimum(gl, n_nodes - 1)][valid]
                s_pack[m, :, ti["s0"]:ti["s0"] + P] = blk
            else:
                j = ti["j"]
                dl = da[m, j]
                v = dl >= 0
                if not v.any():
                    continue
                blk = np.zeros((P, ti["nd"]), np.float32)
                blk[ar[v], dl[v] - ti["dmin"]] = dv[m, j][v]
                s_pack[m, :, ti["s0"]:ti["s0"] + ti["nd"]] = blk

    deg_nat = np.full((NCORES, P, rt), 1.0, np.float32)
    for m in range(NCORES):
        d = np.full(rt * P, 1.0, np.float32)
        d[:ns] = deg[m * ns:(m + 1) * ns]
        deg_nat[m] = d.reshape(rt, P).T

    plan.rng_scols = []
    for rg in range(nrng):
        ts = [ti for ti in plan.tiles if ti["rng"] == rg]
        plan.rng_scols.append((ts[0]["s0"], ts[-1]["s0"] + ts[-1]["nd"])
                              if ts else (0, 0))
    plan.swmax = max((b - a for a, b in plan.rng_scols), default=1)
    plan.nmax = max((s["n"] for s in plan.segs), default=P)

    data = dict(gidx=gidx, s_pack=s_pack, deg_nat=deg_nat, cnts=cnts)
    return plan, data


# ----------------------------------------------------------------------------
# Device program (identical for all cores)
# ----------------------------------------------------------------------------

def _build_nc(n_nodes: int, plan: Plan):
    ns, rt, nch, nrng = plan.ns, plan.rt, plan.nch, plan.nrng
    nc = bacc.Bacc("TRN2", target_bir_lowering=False, debug=False,
                   num_devices=NCORES)

    x_d = nc.dram_tensor("x", [n_nodes, P], BF16, kind="ExternalInput").ap()
    wt_d = nc.dram_tensor("wt", [P, P], F32, kind="ExternalInput").ap()
    gix_d = nc.dram_tensor("gidx", [P, max(1, plan.tot16)], I16,
                           kind="ExternalInput").ap()
    dnat_d = nc.dram_tensor("deg_nat", [P, rt], F32,
                            kind="ExternalInput").ap()
    s_d = nc.dram_tensor("s_pack", [P, plan.stot], BF16,
                         kind="ExternalInput").ap()
    cnt_d = nc.dram_tensor("cnts", [1, max(1, len(plan.segs))],
                           mybir.dt.int32, kind="ExternalInput").ap()
    xs_d = nc.dram_tensor("xself", [nrng * RNG_DTS * P, P], BF16,
                          kind="ExternalInput").ap()
    out_d = nc.dram_tensor("out", [rt * P, P], F32, kind="ExternalOutput").ap()

    pw = RNG_DTS * P
    with tile.TileContext(nc) as tc:
        nc.gpsimd.load_library(library_config.mlp)
        with (
            tc.tile_pool(name="const", bufs=1) as cpool,
            tc.tile_pool(name="gbuf", bufs=3) as gpool,
            tc.tile_pool(name="sfbuf", bufs=2) as sfpool,
            tc.tile_pool(name="sbuf_s", bufs=2) as spool,
            tc.tile_pool(name="fin", bufs=4) as fpool,
            tc.tile_pool(name="pacc", bufs=4, space="PSUM") as papool,
            tc.tile_pool(name="pout", bufs=2, space="PSUM") as popool,
        ):
            wt_sb = cpool.tile([P, P], F32)
            nc.sync.dma_start(out=wt_sb[:], in_=wt_d[:, :])
            gidx_sb = cpool.tile([P, max(1, plan.tot16)], I16)
            nc.sync.dma_start(out=gidx_sb[:], in_=gix_d[:, :])

            dnat_sb = cpool.tile([P, rt], F32)
            nc.sync.dma_start(out=dnat_sb[:], in_=dnat_d[:, :])
            nc.scalar.activation(dnat_sb[:], dnat_sb[:],
                                 mybir.ActivationFunctionType.Sqrt)
            d_nat = cpool.tile([P, rt], F32)
            nc.vector.reciprocal(d_nat[:], dnat_sb[:])

            zcol = cpool.tile([1, P], BF16)
            nc.vector.memset(zcol[:], 0.0)
            zrow = cpool.tile([1, pw], BF16)
            nc.vector.memset(zrow[:], 0.0)

            cnt_sb = cpool.tile([1, max(1, len(plan.segs))], mybir.dt.int32)
            nc.sync.dma_start(out=cnt_sb[:], in_=cnt_d[:, :])
            cnt_regs = [nc.gpsimd.alloc_register(f"cntr{i}") for i in range(4)]

            # zero all gather buffers once up front: afterwards pad slots
            # hold stale but finite gathered values (never NaN bit
            # patterns), which the zero S rows annihilate in the PE
            for _ in range(3):
                gz = gpool.tile([P, plan.nmax], BF16, tag="g")
                nc.vector.memset(gz[:], 0.0)

            self_tiles = {}
            edge_tiles = {}
            for ti in plan.tiles:
                d = self_tiles if ti["kind"] == "self" else edge_tiles
                d.setdefault(ti["rng"], []).append(ti)
            segs_by_rng = {}
            for si, seg in enumerate(plan.segs):
                segs_by_rng.setdefault(seg["rng"], []).append((si, seg))

            for rg in range(nrng):
                sw0, sw1 = plan.rng_scols[rg]
                s_sb = spool.tile([P, plan.swmax], BF16, tag="s_sb")
                nc.sync.dma_start(out=s_sb[:, :sw1 - sw0],
                                  in_=s_d[:, sw0:sw1])

                pt = papool.tile([P, pw], F32, tag="pacc")
                nmm = len(self_tiles.get(rg, [])) + sum(
                    seg["ntiles"] for _, seg in segs_by_rng.get(rg, []))
                nc.tensor.matmul(pt[:], lhsT=zcol[:], rhs=zrow[:],
                                 start=True, stop=(nmm == 0),
                                 skip_group_check=True)
                k = 0

                def mm(lhs, ti):
                    nonlocal k
                    dmin, nd = ti["dmin"], ti["nd"]
                    sa = ti["s0"] - sw0
                    nc.tensor.matmul(
                        pt[:, dmin:dmin + nd], lhsT=lhs,
                        rhs=s_sb[:, sa:sa + nd],
                        start=False, stop=(k == nmm - 1),
                        skip_group_check=True,
                    )
                    k += 1

                sf = sfpool.tile([P, RNG_DTS * P], BF16, tag="sf")
                nc.sync.dma_start(
                    out=sf[:].rearrange("p (t f) -> p t f", f=P),
                    in_=xs_d[rg * pw:(rg + 1) * pw, :].rearrange(
                        "(t p) f -> p t f", p=P))
                for ti in self_tiles.get(rg, []):
                    mm(sf[:, ti["t"] * P:(ti["t"] + 1) * P], ti)

                et = edge_tiles.get(rg, [])
                ei = 0
                for si, seg in segs_by_rng.get(rg, []):
                    jseg, nseg = seg["ntiles"], seg["n"]
                    g = gpool.tile([P, plan.nmax], BF16, tag="g")
                    g3 = g[:, :nseg].rearrange("p (j f) -> p j f", f=P)
                    span = min(CH_SPAN, n_nodes - seg["base"])
                    creg = cnt_regs[si % len(cnt_regs)]
                    nc.gpsimd.reg_load(creg, cnt_sb[0:1, si:si + 1])
                    nc.gpsimd.dma_gather(
                        g3, x_d[seg["base"]:seg["base"] + span, :],
                        gidx_sb[:, seg["t16_0"]:seg["t16_0"] + jseg * 8],
                        nseg, creg, P, single_packet=False,
                    )
                    for jj in range(jseg):
                        ti = et[ei]
                        assert ti["seg"] is seg and ti["jj"] == jj
                        mm(g[:, jj * P:(jj + 1) * P], ti)
                        ei += 1

                for dl in range(min(RNG_DTS, rt - rg * RNG_DTS)):
                    dt = rg * RNG_DTS + dl
                    aggt = fpool.tile([P, P], F32, tag="aggt")
                    nc.vector.tensor_copy(aggt[:], pt[:, dl * P:(dl + 1) * P])
                    op = popool.tile([P, P], F32, tag="op")
                    nc.tensor.matmul(op[:], lhsT=aggt[:], rhs=wt_sb[:],
                                     start=True, stop=True)
                    ob = fpool.tile([P, P], F32, tag="ob")
                    nc.vector.tensor_scalar_mul(ob[:], op[:],
                                                d_nat[:, dt:dt + 1])
                    nc.sync.dma_start(out=out_d[dt * P:(dt + 1) * P, :],
                                      in_=ob[:])
    nc.compile()
    return nc


# ----------------------------------------------------------------------------
# Entry point
# ----------------------------------------------------------------------------

_CACHE: dict = {}


def _prepare(X, W, edge_index):
    X = np.ascontiguousarray(np.asarray(X, dtype=np.float32))
    W = np.asarray(W, dtype=np.float32)
    edge_index = np.asarray(edge_index)
    n = X.shape[0]
    plan, data = _preprocess(edge_index, n)
    key = (n, plan.jtot, plan.stot, tuple(s["n"] for s in plan.segs))
    if key not in _CACHE:
        _CACHE.clear()
        _CACHE[key] = _build_nc(n, plan)
    nc = _CACHE[key]
    wt = np.ascontiguousarray(W.T)
    xs = X.astype(ml_dtypes.bfloat16)
    in_maps = []
    for m in range(NCORES):
        xsf = np.zeros((plan.nrng * RNG_DTS * P, P), ml_dtypes.bfloat16)
        xsf[:plan.ns] = xs[m * plan.ns:(m + 1) * plan.ns]
        in_maps.append({
            "x": np.ascontiguousarray(xs),
            "wt": wt,
            "gidx": np.ascontiguousarray(data["gidx"][m]),
            "deg_nat": np.ascontiguousarray(data["deg_nat"][m]),
            "s_pack": np.ascontiguousarray(data["s_pack"][m]),
            "cnts": np.ascontiguousarray(data["cnts"][m][None, :]),
            "xself": xsf,
        })
    return nc, in_maps, plan


def kernel(X, W, edge_index):
    nc, in_maps, plan = _prepare(X, W, edge_index)
    res = run_bass_kernel_spmd(nc, in_maps, core_ids=list(range(NCORES)))
    ns = plan.ns
    return np.concatenate([res.results[m]["out"][:ns] for m in range(NCORES)],
                          axis=0)


# revision 6
# speedup vs baseline: 1.6576x; 1.0375x over previous
"""GCNConv on 8 Trainium2 NeuronCores (Bass/Tile, SPMD).

out = D^-1/2 (A+I) D^-1/2 (X @ W.T),   deg = in-degree(col) + 1

Formulation (exact in real arithmetic):
    agg[:, r] = sum_{e: dst=r} d[col_e] * X[col_e]   + d[r] * X[r]
    out[r]    = d[r] * (agg[:, r] @ W.T)             (d = deg^-1/2)

Distribution: destinations (rows) sharded across 8 cores (12500 each);
each core processes edges whose destination lands in its shard. X (bf16)
and W are replicated.

Per-core device algorithm (SPMD; per-core index tables padded to a
common max-over-cores structure):
  * Edge slots grouped by (range of RNG_DTS dest-tiles, source chunk of
    25000 rows), sorted by destination, packed densely at 128 slots/tile.
  * One dma_gather (int16 chunk-relative indices) per segment pulls the
    256-byte bf16 X rows; trailing pads use index -1 + a per-core count
    register so padding costs no descriptors. The gather dominates the
    runtime (~4 ns/descriptor marginal on the single SWDGE queue).
  * Routing: per 128-slot tile, a host-built selection matrix S
    (bf16, carrying the d[col] edge values) scatters slots to
    destinations on the PE: psum[feat, dest] += g_tile.T @ S_tile,
    accumulated over a range-wide one-bank PSUM tile [128, 512].
  * Self-loops skip the gather: their rows are the core's own X shard,
    streamed densely per range (static DMA) and routed with diagonal-S
    tiles carrying d[dest].
  * Finalize per dest-tile: matmul with W.T (contraction over features),
    scale by d[dest], DMA out.

Host does index marshaling only (bucketing, sorting, degree counts, S
structure); all floating-point math on X/W runs on device (the bf16
casts of X/S-values are rounding, not arithmetic).
"""

import math

import numpy as np
import ml_dtypes

import concourse.bacc as bacc
import concourse.mybir as mybir
import concourse.tile as tile
from concourse.bass_utils import run_bass_kernel_spmd
from concourse import library_config

NCORES = 8
P = 128
CH_SPAN = 25000          # source rows per gather chunk (int16-indexable)
RNG_DTS = 4              # dest-tiles per range (psum tile = 1 bank)

F32 = mybir.dt.float32
BF16 = mybir.dt.bfloat16
I16 = mybir.dt.int16


class Plan:
    pass


# ----------------------------------------------------------------------------
# Host-side index marshaling
# ----------------------------------------------------------------------------

def _preprocess(edge_index: np.ndarray, n_nodes: int):
    ns = n_nodes // NCORES
    rt = math.ceil(ns / P)
    nch = math.ceil(n_nodes / CH_SPAN)
    nrng = math.ceil(rt / RNG_DTS)
    rw = RNG_DTS * P

    row = np.asarray(edge_index[0]).astype(np.int64)
    col = np.asarray(edge_index[1]).astype(np.int64)
    deg = (np.bincount(col, minlength=n_nodes) + 1).astype(np.float32)
    dinv = deg ** -0.5

    core = row // ns
    cores = []
    for m in range(NCORES):
        sel = core == m
        r_l = row[sel] - m * ns
        c_g = col[sel]
        rg = r_l // rw
        ch = np.minimum(c_g // CH_SPAN, nch - 1)
        order = np.lexsort((c_g, r_l, ch, rg))
        r_l, c_g = r_l[order], c_g[order]
        code = rg[order] * nch + ch[order]
        bounds = np.searchsorted(code, np.arange(nrng * nch + 1))
        cores.append(dict(r_l=r_l, c_g=c_g, bounds=bounds))

    plan = Plan()
    plan.ns, plan.rt, plan.nch, plan.nrng = ns, rt, nch, nrng
    plan.segs = []          # gather segments
    plan.tiles = []         # all S tiles in stream order
    jtot = 0
    for rg in range(nrng):
        ndt = min(RNG_DTS, rt - rg * RNG_DTS)
        for t in range(ndt):
            plan.tiles.append(dict(kind="self", rng=rg, t=t,
                                   dmin=t * P, nd=P))
        for c in range(nch):
            g = rg * nch + c
            ntiles = 0
            for m in range(NCORES):
                b = cores[m]["bounds"]
                ntiles = max(ntiles, (int(b[g + 1] - b[g]) + P - 1) // P)
            if ntiles == 0:
                continue
            seg = dict(base=c * CH_SPAN, t16_0=jtot * 8, n=ntiles * P,
                       j0=jtot, ntiles=ntiles, rng=rg, c=c, g=g)
            plan.segs.append(seg)
            for jj in range(ntiles):
                plan.tiles.append(dict(kind="edge", rng=rg, seg=seg, jj=jj,
                                       j=jtot + jj))
            jtot += ntiles
    plan.jtot = jtot
    plan.tot16 = jtot * 8

    nslots = jtot * P
    gidx = np.zeros((NCORES, P, max(1, plan.tot16)), np.int16)
    dval = np.zeros((NCORES, nslots), np.float32)
    dest_arr = np.full((NCORES, nslots), -1, np.int64)
    cnts = np.zeros((NCORES, max(1, len(plan.segs))), np.int32)
    for m in range(NCORES):
        r_l, c_g, b = cores[m]["r_l"], cores[m]["c_g"], cores[m]["bounds"]
        idx16 = np.full(nslots, -1, np.int16)
        for si, seg in enumerate(plan.segs):
            g = seg["g"]
            lo, hi = int(b[g]), int(b[g + 1])
            n = hi - lo
            if n == 0:
                # still need >= 1 valid index (dummy row 0, zero S row)
                idx16[seg["j0"] * P] = 0
                cnts[m, si] = 1
                continue
            cnts[m, si] = n
            s0 = seg["j0"] * P
            cg = c_g[lo:hi]
            idx16[s0:s0 + n] = (cg - seg["base"]).astype(np.int16)
            dval[m, s0:s0 + n] = dinv[cg]
            dest_arr[m, s0:s0 + n] = r_l[lo:hi] - seg["rng"] * rw
        w = idx16.reshape(max(1, plan.tot16), 16).T
        gidx[m] = np.tile(w, (8, 1))

    # per-tile dest windows (edge tiles: union over cores)
    da = dest_arr.reshape(NCORES, jtot, P)
    da_min = np.where(da < 0, 10 ** 9, da).min(axis=(0, 2))
    da_max = da.max(axis=(0, 2))
    for ti in plan.tiles:
        if ti["kind"] == "edge":
            j = ti["j"]
            ti["dmin"] = int(da_min[j])
            ti["nd"] = int(da_max[j] - da_min[j] + 1)
            assert 1 <= ti["nd"] <= rw
    s0 = 0
    for ti in plan.tiles:
        ti["s0"] = s0
        s0 += ti["nd"]
    plan.stot = s0

    # S matrices (bf16, carrying d[col] / diagonal self d values)
    s_pack = np.zeros((NCORES, P, plan.stot), ml_dtypes.bfloat16)
    ar = np.arange(P)
    dv = dval.reshape(NCORES, jtot, P)
    for m in range(NCORES):
        for ti in plan.tiles:
            if ti["kind"] == "self":
                dt = ti["rng"] * RNG_DTS + ti["t"]
                gl = m * ns + dt * P + ar
                valid = gl < (m + 1) * ns
                blk = np.zeros((P, P), np.float32)
                blk[ar[valid], ar[valid]] = dinv[
                    np.minimum(gl, n_nodes - 1)][valid]
                s_pack[m, :, ti["s0"]:ti["s0"] + P] = blk
            else:
                j = ti["j"]
                dl = da[m, j]
                v = dl >= 0
                if not v.any():
                    continue
                blk = np.zeros((P, ti["nd"]), np.float32)
                blk[ar[v], dl[v] - ti["dmin"]] = dv[m, j][v]
                s_pack[m, :, ti["s0"]:ti["s0"] + ti["nd"]] = blk

    deg_nat = np.full((NCORES, P, rt), 1.0, np.float32)
    for m in range(NCORES):
        d = np.full(rt * P, 1.0, np.float32)
        d[:ns] = deg[m * ns:(m + 1) * ns]
        deg_nat[m] = d.reshape(rt, P).T

    plan.rng_scols = []
    for rg in range(nrng):
        ts = [ti for ti in plan.tiles if ti["rng"] == rg]
        plan.rng_scols.append((ts[0]["s0"], ts[-1]["s0"] + ts[-1]["nd"])
                              if ts else (0, 0))
    plan.swmax = max((b - a for a, b in plan.rng_scols), default=1)
    plan.nmax = max((s["n"] for s in plan.segs), default=P)

    data = dict(gidx=gidx, s_pack=s_pack, deg_nat=deg_nat, cnts=cnts)
    return plan, data


# ----------------------------------------------------------------------------
# Device program (identical for all cores)
# ----------------------------------------------------------------------------

def _build_nc(n_nodes: int, plan: Plan):
    ns, rt, nch, nrng = plan.ns, plan.rt, plan.nch, plan.nrng
    nc = bacc.Bacc("TRN2", target_bir_lowering=False, debug=False,
                   num_devices=NCORES)

    x_d = nc.dram_tensor("x", [n_nodes, P], BF16, kind="ExternalInput").ap()
    wt_d = nc.dram_tensor("wt", [P, P], F32, kind="ExternalInput").ap()
    gix_d = nc.dram_tensor("gidx", [P, max(1, plan.tot16)], I16,
                           kind="ExternalInput").ap()
    dnat_d = nc.dram_tensor("deg_nat", [P, rt], F32,
                            kind="ExternalInput").ap()
    s_d = nc.dram_tensor("s_pack", [P, plan.stot], BF16,
                         kind="ExternalInput").ap()
    cnt_d = nc.dram_tensor("cnts", [1, max(1, len(plan.segs))],
                           mybir.dt.int32, kind="ExternalInput").ap()
    xs_d = nc.dram_tensor("xself", [nrng * RNG_DTS * P, P], BF16,
                          kind="ExternalInput").ap()
    out_d = nc.dram_tensor("out", [rt * P, P], F32, kind="ExternalOutput").ap()

    pw = RNG_DTS * P
    with tile.TileContext(nc) as tc:
        nc.gpsimd.load_library(library_config.mlp)
        with (
            tc.tile_pool(name="const", bufs=1) as cpool,
            tc.tile_pool(name="gbuf", bufs=3) as gpool,
            tc.tile_pool(name="sfbuf", bufs=2) as sfpool,
            tc.tile_pool(name="sbuf_s", bufs=2) as spool,
            tc.tile_pool(name="fin", bufs=4) as fpool,
            tc.tile_pool(name="pacc", bufs=4, space="PSUM") as papool,
            tc.tile_pool(name="pout", bufs=2, space="PSUM") as popool,
        ):
            wt_sb = cpool.tile([P, P], F32)
            nc.sync.dma_start(out=wt_sb[:], in_=wt_d[:, :])
            cnt_sb = cpool.tile([1, max(1, len(plan.segs))], mybir.dt.int32)
            nc.sync.dma_start(out=cnt_sb[:], in_=cnt_d[:, :])
            cnt_regs = [nc.gpsimd.alloc_register(f"cntr{i}") for i in range(4)]

            # load range 0's index columns first so its gathers can start
            # while the rest of the table streams in (subtile deps)
            gidx_sb = cpool.tile([P, max(1, plan.tot16)], I16)
            c1 = max((s["t16_0"] + s["ntiles"] * 8
                      for s in plan.segs if s["rng"] == 0),
                     default=max(1, plan.tot16))
            nc.sync.dma_start(out=gidx_sb[:, :c1], in_=gix_d[:, :c1])
            if c1 < plan.tot16:
                nc.sync.dma_start(out=gidx_sb[:, c1:], in_=gix_d[:, c1:])

            dnat_sb = cpool.tile([P, rt], F32)
            nc.sync.dma_start(out=dnat_sb[:], in_=dnat_d[:, :])
            nc.scalar.activation(dnat_sb[:], dnat_sb[:],
                                 mybir.ActivationFunctionType.Sqrt)
            d_nat = cpool.tile([P, rt], F32)
            nc.vector.reciprocal(d_nat[:], dnat_sb[:])

            zcol = cpool.tile([1, P], BF16)
            nc.vector.memset(zcol[:], 0.0)
            zrow = cpool.tile([1, pw], BF16)
            nc.vector.memset(zrow[:], 0.0)

            # zero all gather buffers once up front: afterwards pad slots
            # hold stale but finite gathered values (never NaN bit
            # patterns), which the zero S rows annihilate in the PE
            for _ in range(3):
                gz = gpool.tile([P, plan.nmax], BF16, tag="g")
                nc.vector.memset(gz[:], 0.0)

            self_tiles = {}
            edge_tiles = {}
            for ti in plan.tiles:
                d = self_tiles if ti["kind"] == "self" else edge_tiles
                d.setdefault(ti["rng"], []).append(ti)
            segs_by_rng = {}
            for si, seg in enumerate(plan.segs):
                segs_by_rng.setdefault(seg["rng"], []).append((si, seg))

            for rg in range(nrng):
                sw0, sw1 = plan.rng_scols[rg]
                s_sb = spool.tile([P, plan.swmax], BF16, tag="s_sb")
                # S streams on the idle ACT engine's DMA queue so it never
                # head-of-line blocks the out/xself traffic on sync
                nc.scalar.dma_start(out=s_sb[:, :sw1 - sw0],
                                    in_=s_d[:, sw0:sw1])

                pt = papool.tile([P, pw], F32, tag="pacc")
                nmm = len(self_tiles.get(rg, [])) + sum(
                    seg["ntiles"] for _, seg in segs_by_rng.get(rg, []))
                nc.tensor.matmul(pt[:], lhsT=zcol[:], rhs=zrow[:],
                                 start=True, stop=(nmm == 0),
                                 skip_group_check=True)
                k = 0

                def mm(lhs, ti):
                    nonlocal k
                    dmin, nd = ti["dmin"], ti["nd"]
                    sa = ti["s0"] - sw0
                    nc.tensor.matmul(
                        pt[:, dmin:dmin + nd], lhsT=lhs,
                        rhs=s_sb[:, sa:sa + nd],
                        start=False, stop=(k == nmm - 1),
                        skip_group_check=True,
                    )
                    k += 1

                sf = sfpool.tile([P, RNG_DTS * P], BF16, tag="sf")
                nc.sync.dma_start(
                    out=sf[:].rearrange("p (t f) -> p t f", f=P),
                    in_=xs_d[rg * pw:(rg + 1) * pw, :].rearrange(
                        "(t p) f -> p t f", p=P))
                for ti in self_tiles.get(rg, []):
                    mm(sf[:, ti["t"] * P:(ti["t"] + 1) * P], ti)

                et = edge_tiles.get(rg, [])
                ei = 0
                for si, seg in segs_by_rng.get(rg, []):
                    jseg, nseg = seg["ntiles"], seg["n"]
                    g = gpool.tile([P, plan.nmax], BF16, tag="g")
                    g3 = g[:, :nseg].rearrange("p (j f) -> p j f", f=P)
                    span = min(CH_SPAN, n_nodes - seg["base"])
                    creg = cnt_regs[si % len(cnt_regs)]
                    nc.gpsimd.reg_load(creg, cnt_sb[0:1, si:si + 1])
                    nc.gpsimd.dma_gather(
                        g3, x_d[seg["base"]:seg["base"] + span, :],
                        gidx_sb[:, seg["t16_0"]:seg["t16_0"] + jseg * 8],
                        nseg, creg, P, single_packet=False,
                    )
                    for jj in range(jseg):
                        ti = et[ei]
                        assert ti["seg"] is seg and ti["jj"] == jj
                        mm(g[:, jj * P:(jj + 1) * P], ti)
                        ei += 1

                for dl in range(min(RNG_DTS, rt - rg * RNG_DTS)):
                    dt = rg * RNG_DTS + dl
                    aggt = fpool.tile([P, P], F32, tag="aggt")
                    nc.vector.tensor_copy(aggt[:], pt[:, dl * P:(dl + 1) * P])
                    op = popool.tile([P, P], F32, tag="op")
                    nc.tensor.matmul(op[:], lhsT=aggt[:], rhs=wt_sb[:],
                                     start=True, stop=True)
                    ob = fpool.tile([P, P], F32, tag="ob")
                    nc.vector.tensor_scalar_mul(ob[:], op[:],
                                                d_nat[:, dt:dt + 1])
                    nc.sync.dma_start(out=out_d[dt * P:(dt + 1) * P, :],
                                      in_=ob[:])
    nc.compile()
    return nc


# ----------------------------------------------------------------------------
# Entry point
# ----------------------------------------------------------------------------

_CACHE: dict = {}


def _prepare(X, W, edge_index):
    X = np.ascontiguousarray(np.asarray(X, dtype=np.float32))
    W = np.asarray(W, dtype=np.float32)
    edge_index = np.asarray(edge_index)
    n = X.shape[0]
    plan, data = _preprocess(edge_index, n)
    key = (n, plan.jtot, plan.stot, tuple(s["n"] for s in plan.segs))
    if key not in _CACHE:
        _CACHE.clear()
        _CACHE[key] = _build_nc(n, plan)
    nc = _CACHE[key]
    wt = np.ascontiguousarray(W.T)
    xs = X.astype(ml_dtypes.bfloat16)
    in_maps = []
    for m in range(NCORES):
        xsf = np.zeros((plan.nrng * RNG_DTS * P, P), ml_dtypes.bfloat16)
        xsf[:plan.ns] = xs[m * plan.ns:(m + 1) * plan.ns]
        in_maps.append({
            "x": np.ascontiguousarray(xs),
            "wt": wt,
            "gidx": np.ascontiguousarray(data["gidx"][m]),
            "deg_nat": np.ascontiguousarray(data["deg_nat"][m]),
            "s_pack": np.ascontiguousarray(data["s_pack"][m]),
            "cnts": np.ascontiguousarray(data["cnts"][m][None, :]),
            "xself": xsf,
        })
    return nc, in_maps, plan


def kernel(X, W, edge_index):
    nc, in_maps, plan = _prepare(X, W, edge_index)
    res = run_bass_kernel_spmd(nc, in_maps, core_ids=list(range(NCORES)))
    ns = plan.ns
    return np.concatenate([res.results[m]["out"][:ns] for m in range(NCORES)],
                          axis=0)
